# revision 5
# baseline (speedup 1.0000x reference)
"""AFT (attention-free transformer) block on 8 TRN2 NeuronCores.

Reference computation (T=2048, B=4, D=1024):
    qkv = data @ W_qkv + b_qkv ; q,k,v = split(qkv)
    num = exp(pb - max_pb) @ (exp(k - max_k) * v)    (contraction over key pos j)
    den = exp(pb - max_pb) @ exp(k - max_k)
    out = (sigmoid(q) * num / den) @ W_out + b_out

The max_pb / max_k shifts cancel exactly in num/den, and the value ranges here
(|k| <~ 4, |pb| <~ 0.12) are far from overflow, so the kernel drops them.

Sharding: sequence-parallel over the query axis i. Core c owns i in
[c*256, (c+1)*256). Each core computes q/k/v for its own 256 query rows,
all-gathers exp(k) and exp(k)*v (bf16) across the 8 cores so the full j axis
is local, then does its num/den rows and the output projection.

Layouts (per core, tokens = local (i, b) pairs, 1024 of them):
  dataT  [1024 d_in, 1024 tok]   (host pre-transposed, bf16)
  qT     [1024 d,    1024 tok]   feature-major  (lhsT = W_q)
  k, v   [1024 tok,  1024 d]     token-major    (lhsT = dataT)
  ek/ekv [tok, d] -> AllGather -> [8 ranks][2][1024 tok][1024 d] bf16
  pbT    [2048 j, 256 i]         (host pre-transposed slice, bf16)
  num/den[d, i] feature-major    (lhsT = ekv/ek, rhs = exp(pbT))
  yT     [d, i]  = sigmoid(qT) * num * recip(den)
  out    [i, d_out] token-major  (lhsT = yT, rhs = W_out)
No on-chip transposes anywhere.
"""

import numpy as np
import ml_dtypes

from concourse import bacc, bass, mybir, tile
from concourse.bass_utils import run_bass_kernel_spmd

BF16 = mybir.dt.bfloat16
F32 = mybir.dt.float32
AF = mybir.ActivationFunctionType

N_CORES = 8
T, B, D = 2048, 4, 1024
TL = T // N_CORES          # 256 local query rows
TOK = TL * B               # 1024 local tokens
KT = D // 128              # 8 contraction tiles for d
JT = T // 128              # 16 j tiles

_cache = {}


def build(with_qkv_bias: bool, with_out_bias: bool):
    nc = bacc.Bacc(None, target_bir_lowering=False)

    dataT_d = nc.dram_tensor("dataT", [D, TOK], BF16, kind="ExternalInput")
    wq_d = nc.dram_tensor("wq", [D, D], BF16, kind="ExternalInput")
    wkv_d = nc.dram_tensor("wkv", [D, 2 * D], BF16, kind="ExternalInput")
    pbT_d = nc.dram_tensor("pbT", [T, TL], BF16, kind="ExternalInput")
    wout_d = nc.dram_tensor("wout", [D, D], BF16, kind="ExternalInput")
    out_d = nc.dram_tensor("out", [TOK, D], F32, kind="ExternalOutput")
    if with_qkv_bias:
        bq_d = nc.dram_tensor("bq", [D, 1], F32, kind="ExternalInput")
        bkv_d = nc.dram_tensor("bkv", [1, 2 * D], BF16, kind="ExternalInput")
    if with_out_bias:
        bout_d = nc.dram_tensor("bout", [1, D], BF16, kind="ExternalInput")

    with tile.TileContext(nc) as tc:
        with (
            tc.tile_pool(name="persist", bufs=1) as pp,
            tc.tile_pool(name="psum_mm", bufs=4, space="PSUM") as psmm,
            tc.tile_pool(name="psum_nd", bufs=4, space="PSUM") as psnd,
            tc.tile_pool(name="dram", bufs=1, space="DRAM") as dram,
        ):
            # ---- persistent SBUF tensors ----
            wout = [pp.tile([128, D], BF16, name=f"wout{k}", tag=f"wout{k}")
                    for k in range(KT)]
            for k in range(KT):
                nc.sync.dma_start(wout[k][:], wout_d[k * 128:(k + 1) * 128, :])
            pbe = [pp.tile([128, TL], BF16, name=f"pbe{t}", tag=f"pbe{t}")
                   for t in range(JT)]
            sigq = [pp.tile([128, TOK], BF16, name=f"sigq{m}", tag=f"sigq{m}")
                    for m in range(KT)]
            if with_qkv_bias:
                ones1 = pp.tile([1, 128], BF16, name="ones1", tag="ones1")
                nc.gpsimd.memset(ones1[:], 1.0)
                bkv = pp.tile([1, 2 * D], BF16, name="bkv", tag="bkv")
                nc.sync.dma_start(bkv[:], bkv_d[:])
                bq = [pp.tile([128, 1], F32, name=f"bq{m}", tag=f"bq{m}")
                      for m in range(KT)]
                for m in range(KT):
                    nc.sync.dma_start(bq[m][:], bq_d[m * 128:(m + 1) * 128, :])
            if with_out_bias:
                if not with_qkv_bias:
                    ones1 = pp.tile([1, 128], BF16, name="ones1", tag="ones1")
                    nc.gpsimd.memset(ones1[:], 1.0)
                bout = pp.tile([1, D], BF16, name="bout", tag="bout")
                nc.sync.dma_start(bout[:], bout_d[:])

            cc_in = dram.tile([2 * TOK, D], BF16, name="cc_in")
            cc_out = dram.tile([N_CORES * 2 * TOK, D], BF16, name="cc_out",
                               addr_space="Shared")

            # ---- phase A: qkv projection ----
            with tc.tile_pool(name="phaseA", bufs=1) as pa:
                dataT = [pa.tile([128, TOK], BF16, name=f"dataT{k}", tag=f"dataT{k}")
                         for k in range(KT)]
                for k in range(KT):
                    nc.sync.dma_start(dataT[k][:], dataT_d[k * 128:(k + 1) * 128, :])
                wkv = [pa.tile([128, 2 * D], BF16, name=f"wkv{k}", tag=f"wkv{k}")
                       for k in range(KT)]
                for k in range(KT):
                    nc.sync.dma_start(wkv[k][:], wkv_d[k * 128:(k + 1) * 128, :])
                wq = [pa.tile([128, D], BF16, name=f"wq{k}", tag=f"wq{k}")
                      for k in range(KT)]
                for k in range(KT):
                    nc.sync.dma_start(wq[k][:], wq_d[k * 128:(k + 1) * 128, :])

                # k/v (token-major) -> exp(k), exp(k)*v -> cc_in
                for m in range(KT):  # token tile
                    ek = pa.tile([128, D], BF16, name=f"ek{m}", tag="ek", bufs=2)
                    vv = pa.tile([128, D], BF16, name=f"vv{m}", tag="vv", bufs=2)
                    ekv = pa.tile([128, D], BF16, name=f"ekv{m}", tag="ekv", bufs=2)
                    for n in range(4):  # 512-wide chunks of [k | v]
                        ps = psmm.tile([128, 512], F32, name=f"ps_kv{m}_{n}", tag="mm")
                        for k in range(KT):
                            nc.tensor.matmul(
                                ps[:], dataT[k][:, m * 128:(m + 1) * 128],
                                wkv[k][:, n * 512:(n + 1) * 512],
                                start=(k == 0),
                                stop=(k == KT - 1 and not with_qkv_bias),
                            )
                        if with_qkv_bias:
                            nc.tensor.matmul(
                                ps[:], ones1[:],
                                bkv[:, n * 512:(n + 1) * 512],
                                start=False, stop=True,
                            )
                        if n < 2:  # k chunk -> exp
                            nc.scalar.activation(
                                ek[:, n * 512:(n + 1) * 512], ps[:], AF.Exp)
                        else:      # v chunk -> copy
                            nc.vector.tensor_copy(
                                vv[:, (n - 2) * 512:(n - 1) * 512], ps[:])
                    nc.vector.tensor_mul(ekv[:], ek[:], vv[:])
                    nc.sync.dma_start(cc_in[m * 128:(m + 1) * 128, :], ek[:])
                    nc.sync.dma_start(cc_in[TOK + m * 128:TOK + (m + 1) * 128, :],
                                      ekv[:])

                # all-gather ek|ekv across the 8 cores
                nc.gpsimd.collective_compute(
                    "AllGather", mybir.AluOpType.bypass,
                    replica_groups=[list(range(N_CORES))],
                    ins=[cc_in[:].opt()], outs=[cc_out[:].opt()],
                )

                # qT (feature-major) + sigmoid — overlaps the collective
                for m in range(KT):  # d_q tile
                    for n in range(2):  # 512-wide token chunks
                        ps = psmm.tile([128, 512], F32, name=f"ps_q{m}_{n}", tag="mm")
                        for k in range(KT):
                            nc.tensor.matmul(
                                ps[:], wq[k][:, m * 128:(m + 1) * 128],
                                dataT[k][:, n * 512:(n + 1) * 512],
                                start=(k == 0), stop=(k == KT - 1),
                            )
                        nc.scalar.activation(
                            sigq[m][:, n * 512:(n + 1) * 512], ps[:], AF.Sigmoid,
                            bias=(bq[m][:] if with_qkv_bias else 0.0))

                # exp(pbT) — also overlaps the collective
                for t in range(JT):
                    praw = pa.tile([128, TL], BF16, name=f"praw{t}", tag="praw",
                                   bufs=2)
                    nc.sync.dma_start(praw[:], pbT_d[t * 128:(t + 1) * 128, :])
                    nc.scalar.activation(pbe[t][:], praw[:], AF.Exp)

            # ---- phase B: num/den + y + output projection ----
            # gathered view: row = r*2048 + h*1024 + (x*128+p)*4 + b
            ccv = cc_out[:].rearrange("(r h x p b) d -> r b p h x d",
                                      r=N_CORES, h=2, x=2, p=128, b=B)
            out_v = out_d[:].rearrange("(m p b) d -> m b p d", m=2, p=128, b=B)
            with tc.tile_pool(name="phaseB", bufs=1) as pb_pool:
                for b in range(B):
                    ekg = []
                    for r in range(N_CORES):
                        g = pb_pool.tile([128, 4096], BF16, name=f"ekg{b}_{r}",
                                         tag="ekg", bufs=9)
                        gv = g[:].rearrange("p (h x d) -> p h x d", h=2, x=2)
                        nc.sync.dma_start(gv, ccv[r, b])
                        ekg.append(g)

                    yT = []
                    for m in range(KT):  # d tile
                        pnum = psnd.tile([128, TL], F32, name=f"pnum{b}_{m}",
                                         tag="nd")
                        pden = psnd.tile([128, TL], F32, name=f"pden{b}_{m}",
                                         tag="nd")
                        for t in range(JT):
                            r, x = t // 2, t % 2
                            # free layout of ekg: h*2048 + x*1024 + d
                            nc.tensor.matmul(
                                pnum[:],
                                ekg[r][:, 2048 + x * 1024 + m * 128:
                                       2048 + x * 1024 + (m + 1) * 128],
                                pbe[t][:],
                                start=(t == 0), stop=(t == JT - 1),
                            )
                        for t in range(JT):
                            r, x = t // 2, t % 2
                            nc.tensor.matmul(
                                pden[:],
                                ekg[r][:, x * 1024 + m * 128:
                                       x * 1024 + (m + 1) * 128],
                                pbe[t][:],
                                start=(t == 0), stop=(t == JT - 1),
                            )
                        rec = pb_pool.tile([128, TL], F32, name=f"rec{b}_{m}",
                                           tag="rec", bufs=3)
                        tmp = pb_pool.tile([128, TL], F32, name=f"tmp{b}_{m}",
                                           tag="tmp", bufs=3)
                        y = pb_pool.tile([128, TL], BF16, name=f"yT{b}_{m}",
                                         tag=f"yT{m}", bufs=2)
                        nc.vector.reciprocal(rec[:], pden[:])
                        nc.vector.tensor_mul(tmp[:], pnum[:], rec[:])
                        sq = sigq[m][:].rearrange("p (i b) -> p i b", b=B)[:, :, b]
                        nc.vector.tensor_mul(y[:], tmp[:], sq)
                        yT.append(y)

                    for m2 in range(2):  # output i tile
                        for n in range(2):  # 512-wide d_out chunks
                            po = psmm.tile([128, 512], F32, name=f"po{b}_{m2}_{n}",
                                           tag="mm")
                            for k in range(KT):
                                nc.tensor.matmul(
                                    po[:], yT[k][:, m2 * 128:(m2 + 1) * 128],
                                    wout[k][:, n * 512:(n + 1) * 512],
                                    start=(k == 0),
                                    stop=(k == KT - 1 and not with_out_bias),
                                )
                            if with_out_bias:
                                nc.tensor.matmul(
                                    po[:], ones1[:], bout[:, n * 512:(n + 1) * 512],
                                    start=False, stop=True,
                                )
                            osb = pb_pool.tile([128, 512], F32,
                                               name=f"osb{b}_{m2}_{n}", tag="osb",
                                               bufs=4)
                            nc.vector.tensor_copy(osb[:], po[:])
                            nc.sync.dma_start(
                                out_v[m2, b][:, n * 512:(n + 1) * 512], osb[:])

    nc.compile()
    return nc


def _prep_inputs(data, W_qkv, b_qkv, pos_bias_param, W_out, b_out):
    bf = ml_dtypes.bfloat16
    data = np.asarray(data, np.float32)
    W_qkv = np.asarray(W_qkv, np.float32)
    b_qkv = np.asarray(b_qkv, np.float32)
    pos_bias_param = np.asarray(pos_bias_param, np.float32)
    W_out = np.asarray(W_out, np.float32)
    b_out = np.asarray(b_out, np.float32)

    with_qkv_bias = bool(np.any(b_qkv))
    with_out_bias = bool(np.any(b_out))

    wq = np.ascontiguousarray(W_qkv[:, :D]).astype(bf)
    wkv = np.ascontiguousarray(W_qkv[:, D:]).astype(bf)
    wout = W_out.astype(bf)
    pbT = np.ascontiguousarray(pos_bias_param.T)  # [j, i]

    in_maps = []
    for c in range(N_CORES):
        sl = slice(c * TL, (c + 1) * TL)
        dT = np.ascontiguousarray(
            data[sl].reshape(TOK, D).T).astype(bf)          # [d_in, tok]
        pbT_c = np.ascontiguousarray(pbT[:, sl]).astype(bf)  # [j, i_loc]
        m = {"dataT": dT, "wq": wq, "wkv": wkv, "pbT": pbT_c, "wout": wout}
        if with_qkv_bias:
            m["bq"] = np.ascontiguousarray(b_qkv[:D]).reshape(D, 1)
            m["bkv"] = np.ascontiguousarray(b_qkv[D:]).reshape(1, 2 * D).astype(bf)
        if with_out_bias:
            m["bout"] = b_out.reshape(1, D).astype(bf)
        in_maps.append(m)
    return in_maps, with_qkv_bias, with_out_bias


def run(data, W_qkv, b_qkv, pos_bias_param, W_out, b_out, **spmd_kwargs):
    in_maps, wb, ob = _prep_inputs(data, W_qkv, b_qkv, pos_bias_param, W_out, b_out)
    key = (wb, ob)
    if key not in _cache:
        _cache[key] = build(wb, ob)
    nc = _cache[key]
    res = run_bass_kernel_spmd(nc, in_maps, core_ids=list(range(N_CORES)),
                               **spmd_kwargs)
    out = np.concatenate([r["out"] for r in res.results], axis=0)
    return out.reshape(T, B, D), res


def kernel(data, W_qkv, b_qkv, pos_bias_param, W_out, b_out):
    out, _ = run(data, W_qkv, b_qkv, pos_bias_param, W_out, b_out)
    return out


# revision 6
# speedup vs baseline: 1.1576x; 1.1576x over previous
"""AFT (attention-free transformer) block on 8 TRN2 NeuronCores — v2.

Reference computation (T=2048, B=4, D=1024):
    qkv = data @ W_qkv + b_qkv ; q,k,v = split(qkv)
    num = exp(pb - max_pb) @ (exp(k - max_k) * v)    (contraction over key pos j)
    den = exp(pb - max_pb) @ exp(k - max_k)
    out = (sigmoid(q) * num / den) @ W_out + b_out
The max shifts cancel exactly in num/den and value ranges are tiny, so the
kernel drops them.

Sharding: sequence-parallel over the query axis i; core c owns i in
[c*256,(c+1)*256). Each core computes q/k/v for its own rows, all-gathers
exp(k) and exp(k)*v (bf16, two pipelined chunks), then computes its num/den
rows and the output projection.

v2 structure (vs v1): every matmul loop reuses one stationary (lhsT) load
for 2-4 N=512 moving passes (the compile config runs with ldw-opt off, so
LDWEIGHTS serialize with matmuls); num/den keep exp(pbT) stationary (shared
across num, den and both d-chunks); y comes out token-major and is
PE-transposed (64x 128x128) for the output projection; sigmoid(q) is bounced
through DRAM to get the batch-separated layout; the AllGather is split into
two chunks so it pipelines with qkv production and num/den consumption.
"""

import numpy as np
import ml_dtypes

from concourse import bacc, bass, mybir, tile
from concourse.bass_utils import run_bass_kernel_spmd
from concourse.masks import make_identity

BF16 = mybir.dt.bfloat16
F32 = mybir.dt.float32
AF = mybir.ActivationFunctionType

N_CORES = 8
T, B, D = 2048, 4, 1024
TL = T // N_CORES          # 256 local query rows
TOK = TL * B               # 1024 local tokens
KT = D // 128              # 8 contraction tiles for d
JT = T // 128              # 16 j tiles

_cache = {}


def build(with_qkv_bias: bool, with_out_bias: bool):
    nc = bacc.Bacc(None, target_bir_lowering=False)

    dataT_d = nc.dram_tensor("dataT", [D, TOK], BF16, kind="ExternalInput")
    wqkv_d = nc.dram_tensor("wqkv", [D, 3 * D], BF16, kind="ExternalInput")
    pbT_d = nc.dram_tensor("pbT", [T, TL], BF16, kind="ExternalInput")
    wout_d = nc.dram_tensor("wout", [D, D], BF16, kind="ExternalInput")
    out_d = nc.dram_tensor("out", [TOK, D], F32, kind="ExternalOutput")
    if with_qkv_bias:
        bqkv_d = nc.dram_tensor("bqkv", [1, 3 * D], BF16, kind="ExternalInput")
    if with_out_bias:
        bout_d = nc.dram_tensor("bout", [1, D], BF16, kind="ExternalInput")

    with tile.TileContext(nc) as tc:
        with (
            tc.tile_pool(name="persist", bufs=1) as pp,
            tc.tile_pool(name="psum_mm", bufs=6, space="PSUM") as psmm,
            tc.tile_pool(name="psum_tr", bufs=2, space="PSUM") as pstr,
            tc.tile_pool(name="dram", bufs=1, space="DRAM") as dram,
        ):
            # ---- persistent SBUF tensors ----
            ident = pp.tile([128, 128], BF16, name="ident", tag="ident")
            make_identity(nc, ident[:])
            wout = [pp.tile([128, D], BF16, name=f"wout{k}", tag=f"wout{k}")
                    for k in range(KT)]
            pbe = [pp.tile([128, TL], BF16, name=f"pbe{t}", tag=f"pbe{t}")
                   for t in range(JT)]
            need_bias_ones = with_qkv_bias or with_out_bias
            if need_bias_ones:
                ones1 = pp.tile([1, 128], BF16, name="ones1", tag="ones1")
                nc.gpsimd.memset(ones1[:], 1.0)
            if with_qkv_bias:
                bqkv = pp.tile([1, 3 * D], BF16, name="bqkv", tag="bqkv")
                nc.sync.dma_start(bqkv[:], bqkv_d[:])
            if with_out_bias:
                bout = pp.tile([1, D], BF16, name="bout", tag="bout")
                nc.sync.dma_start(bout[:], bout_d[:])

            # collective bounce buffers: two token-half chunks
            cc_in = [dram.tile([TOK, D], BF16, name=f"cc_in{x}") for x in range(2)]
            cc_out = [dram.tile([N_CORES * TOK, D], BF16, name=f"cc_out{x}",
                                addr_space="Shared") for x in range(2)]
            sigq_d = dram.tile([TOK, D], BF16, name="sigq_d")

            # ---- phase A: fused qkv projection ----
            with tc.tile_pool(name="phaseA", bufs=1) as pa:
                dataT = [pa.tile([128, TOK], BF16, name=f"dataT{k}",
                                 tag=f"dataT{k}") for k in range(KT)]
                wqkv = [pa.tile([128, 3 * D], BF16, name=f"wqkv{k}",
                                tag=f"wqkv{k}") for k in range(KT)]
                # interleave so matmuls can start after the first k pair lands
                for k in range(KT):
                    nc.sync.dma_start(dataT[k][:], dataT_d[k * 128:(k + 1) * 128, :])
                    nc.sync.dma_start(wqkv[k][:], wqkv_d[k * 128:(k + 1) * 128, :])
                for k in range(KT):
                    nc.sync.dma_start(wout[k][:], wout_d[k * 128:(k + 1) * 128, :])

                for m in range(KT):  # token tile
                    sq = pa.tile([128, D], BF16, name=f"sq{m}", tag="sq", bufs=2)
                    ek = pa.tile([128, D], BF16, name=f"ek{m}", tag="ek", bufs=2)
                    vv = pa.tile([128, D], BF16, name=f"vv{m}", tag="vv", bufs=2)
                    ekv = pa.tile([128, D], BF16, name=f"ekv{m}", tag="ekv", bufs=2)
                    for g in range(2):  # chunk group: 3 psum banks each
                        ps = [psmm.tile([128, 512], F32, name=f"ps{m}_{g}_{i}",
                                        tag="mm") for i in range(3)]
                        for k in range(KT):
                            for i in range(3):
                                n = g * 3 + i
                                nc.tensor.matmul(
                                    ps[i][:], dataT[k][:, m * 128:(m + 1) * 128],
                                    wqkv[k][:, n * 512:(n + 1) * 512],
                                    start=(k == 0),
                                    stop=(k == KT - 1 and not with_qkv_bias),
                                )
                        if with_qkv_bias:
                            for i in range(3):
                                n = g * 3 + i
                                nc.tensor.matmul(
                                    ps[i][:], ones1[:],
                                    bqkv[:, n * 512:(n + 1) * 512],
                                    start=False, stop=True,
                                )
                        for i in range(3):
                            n = g * 3 + i
                            if n < 2:      # q chunk
                                nc.scalar.activation(
                                    sq[:, n * 512:(n + 1) * 512], ps[i][:],
                                    AF.Sigmoid)
                            elif n < 4:    # k chunk
                                nc.scalar.activation(
                                    ek[:, (n - 2) * 512:(n - 1) * 512], ps[i][:],
                                    AF.Exp)
                            else:          # v chunk
                                nc.vector.tensor_copy(
                                    vv[:, (n - 4) * 512:(n - 3) * 512], ps[i][:])
                    nc.vector.tensor_mul(ekv[:], ek[:], vv[:])
                    nc.sync.dma_start(sigq_d[m * 128:(m + 1) * 128, :], sq[:])
                    # chunk x = m//4 holds token rows [x*512,(x+1)*512):
                    # layout [ek half | ekv half]
                    x, mm = m // 4, m % 4
                    nc.sync.dma_start(
                        cc_in[x][mm * 128:(mm + 1) * 128, :], ek[:])
                    nc.sync.dma_start(
                        cc_in[x][512 + mm * 128:512 + (mm + 1) * 128, :], ekv[:])
                    if m == 3:
                        nc.gpsimd.collective_compute(
                            "AllGather", mybir.AluOpType.bypass,
                            replica_groups=[list(range(N_CORES))],
                            ins=[cc_in[0][:].opt()], outs=[cc_out[0][:].opt()],
                        )
                nc.gpsimd.collective_compute(
                    "AllGather", mybir.AluOpType.bypass,
                    replica_groups=[list(range(N_CORES))],
                    ins=[cc_in[1][:].opt()], outs=[cc_out[1][:].opt()],
                )

                # exp(pbT) — overlaps the collectives
                for t in range(JT):
                    praw = pa.tile([128, TL], BF16, name=f"praw{t}", tag="praw",
                                   bufs=2)
                    nc.sync.dma_start(praw[:], pbT_d[t * 128:(t + 1) * 128, :])
                    nc.scalar.activation(pbe[t][:], praw[:], AF.Exp)

            # ---- phase B: num/den + y + output projection ----
            # chunk x gathered rows: r*1024 + h*512 + p*4 + b  (p = local j in tile)
            ccv = [cc_out[x][:].rearrange("(r h p b) d -> r b p h d",
                                          r=N_CORES, h=2, p=128, b=B)
                   for x in range(2)]
            sqv = sigq_d[:].rearrange("(m p b) d -> m b p d", m=2, p=128, b=B)
            out_v = out_d[:].rearrange("(m p b) d -> m b p d", m=2, p=128, b=B)

            with tc.tile_pool(name="phaseB", bufs=1) as pbp:
                for b in range(B):
                    # gathered tiles: ekg[x][r] = [128, (h d)] ; j tile t = 2r+x
                    ekg = [[None] * N_CORES for _ in range(2)]
                    for x in range(2):
                        for r in range(N_CORES):
                            g = pbp.tile([128, 2048], BF16, name=f"ekg{b}_{x}_{r}",
                                         tag="ekg", bufs=18)
                            gv = g[:].rearrange("p (h d) -> p h d", h=2)
                            nc.sync.dma_start(gv, ccv[x][r, b])
                            ekg[x][r] = g
                    sqb = [pbp.tile([128, D], BF16, name=f"sqb{b}_{m2}",
                                    tag="sqb", bufs=4) for m2 in range(2)]
                    for m2 in range(2):
                        nc.sync.dma_start(sqb[m2][:], sqv[m2, b])

                    yT = [pbp.tile([128, TL], BF16, name=f"yT{b}_{k}",
                                   tag=f"yT{k}", bufs=2) for k in range(KT)]
                    for m2 in range(2):  # query i tile
                        pn = [psmm.tile([128, 512], F32, name=f"pn{b}_{m2}_{i}",
                                        tag="mm") for i in range(2)]
                        pd = [psmm.tile([128, 512], F32, name=f"pd{b}_{m2}_{i}",
                                        tag="mm") for i in range(2)]
                        # accumulate chunk-0 j tiles first (even t), then chunk 1
                        order = [(x, r) for x in range(2) for r in range(N_CORES)]
                        for idx, (x, r) in enumerate(order):
                            t = 2 * r + x
                            first, last = idx == 0, idx == len(order) - 1
                            for i in range(2):  # d chunk
                                nc.tensor.matmul(
                                    pn[i][:], pbe[t][:, m2 * 128:(m2 + 1) * 128],
                                    ekg[x][r][:, D + i * 512:D + (i + 1) * 512],
                                    start=first, stop=last)
                                nc.tensor.matmul(
                                    pd[i][:], pbe[t][:, m2 * 128:(m2 + 1) * 128],
                                    ekg[x][r][:, i * 512:(i + 1) * 512],
                                    start=first, stop=last)
                        y = pbp.tile([128, D], BF16, name=f"y{b}_{m2}", tag="y",
                                     bufs=3)
                        for i in range(2):
                            rec = pbp.tile([128, 512], F32, name=f"rec{b}{m2}{i}",
                                           tag="rec", bufs=3)
                            tmp = pbp.tile([128, 512], F32, name=f"tmp{b}{m2}{i}",
                                           tag="tmp", bufs=3)
                            nc.vector.reciprocal_approx_fast(rec[:], pd[i][:])
                            nc.vector.tensor_mul(tmp[:], pn[i][:], rec[:])
                            nc.vector.tensor_mul(
                                y[:, i * 512:(i + 1) * 512], tmp[:],
                                sqb[m2][:, i * 512:(i + 1) * 512])
                        # transpose y [i, d] -> yT [d, i] via PE, 128x128 blocks
                        for k in range(KT):
                            pt = pstr.tile([128, 128], BF16, name=f"pt{b}{m2}{k}",
                                           tag="tr")
                            nc.tensor.transpose(
                                pt[:], y[:, k * 128:(k + 1) * 128], ident[:])
                            nc.vector.tensor_copy(
                                yT[k][:, m2 * 128:(m2 + 1) * 128], pt[:])

                    po = [psmm.tile([128, 512], F32, name=f"po{b}_{m2}_{n}",
                                    tag="mm")
                          for m2 in range(2) for n in range(2)]
                    for k in range(KT):
                        for m2 in range(2):
                            for n in range(2):
                                nc.tensor.matmul(
                                    po[m2 * 2 + n][:],
                                    yT[k][:, m2 * 128:(m2 + 1) * 128],
                                    wout[k][:, n * 512:(n + 1) * 512],
                                    start=(k == 0),
                                    stop=(k == KT - 1 and not with_out_bias))
                    if with_out_bias:
                        for m2 in range(2):
                            for n in range(2):
                                nc.tensor.matmul(
                                    po[m2 * 2 + n][:], ones1[:],
                                    bout[:, n * 512:(n + 1) * 512],
                                    start=False, stop=True)
                    for m2 in range(2):
                        for n in range(2):
                            osb = pbp.tile([128, 512], F32,
                                           name=f"osb{b}_{m2}_{n}", tag="osb",
                                           bufs=4)
                            nc.vector.tensor_copy(osb[:], po[m2 * 2 + n][:])
                            nc.sync.dma_start(
                                out_v[m2, b][:, n * 512:(n + 1) * 512], osb[:])

    nc.compile()
    return nc


def _prep_inputs(data, W_qkv, b_qkv, pos_bias_param, W_out, b_out):
    bf = ml_dtypes.bfloat16
    data = np.asarray(data, np.float32)
    W_qkv = np.asarray(W_qkv, np.float32)
    b_qkv = np.asarray(b_qkv, np.float32)
    pos_bias_param = np.asarray(pos_bias_param, np.float32)
    W_out = np.asarray(W_out, np.float32)
    b_out = np.asarray(b_out, np.float32)

    with_qkv_bias = bool(np.any(b_qkv))
    with_out_bias = bool(np.any(b_out))

    wqkv = W_qkv.astype(bf)
    wout = W_out.astype(bf)
    pbT = np.ascontiguousarray(pos_bias_param.T)  # [j, i]

    in_maps = []
    for c in range(N_CORES):
        sl = slice(c * TL, (c + 1) * TL)
        dT = np.ascontiguousarray(
            data[sl].reshape(TOK, D).T).astype(bf)          # [d_in, tok]
        pbT_c = np.ascontiguousarray(pbT[:, sl]).astype(bf)  # [j, i_loc]
        m = {"dataT": dT, "wqkv": wqkv, "pbT": pbT_c, "wout": wout}
        if with_qkv_bias:
            m["bqkv"] = b_qkv.reshape(1, 3 * D).astype(bf)
        if with_out_bias:
            m["bout"] = b_out.reshape(1, D).astype(bf)
        in_maps.append(m)
    return in_maps, with_qkv_bias, with_out_bias


def run(data, W_qkv, b_qkv, pos_bias_param, W_out, b_out, **spmd_kwargs):
    in_maps, wb, ob = _prep_inputs(data, W_qkv, b_qkv, pos_bias_param, W_out, b_out)
    key = (wb, ob)
    if key not in _cache:
        _cache[key] = build(wb, ob)
    nc = _cache[key]
    res = run_bass_kernel_spmd(nc, in_maps, core_ids=list(range(N_CORES)),
                               **spmd_kwargs)
    out = np.concatenate([r["out"] for r in res.results], axis=0)
    return out.reshape(T, B, D), res


def kernel(data, W_qkv, b_qkv, pos_bias_param, W_out, b_out):
    out, _ = run(data, W_qkv, b_qkv, pos_bias_param, W_out, b_out)
    return out
